# revision 1
# baseline (speedup 1.0000x reference)
# Multi-head attention (B=4, N=2048, D=1024, H=16, DH=64, OUT=1024) on 8 TRN2 NeuronCores.
#
# Sharding: 4 groups x 2 cores. Core c -> batch c//2, head-half c%2 (8 heads).
# Wq/Wk/Wv column-split per head group, Wo row-split; host sums the two
# partial outputs per batch (row-parallel unshard). bo folded in on even cores.
#
# Per-core kernel (all compute bf16 inputs, fp32 PSUM accumulation):
#   qT/kT projections in transposed layout [head_cols(128=2 heads), rows],
#   v projection in natural layout augmented with a ones column (M=65) so the
#   attention AV matmul emits softmax denominators for free.
#   scoresT [krow, qrow] via row-tiled K=64 matmul pairs (2 heads concurrent
#   in PE row groups 0/64, measured ~2x on HW). exp on ScalarE over
#   [128,1024] double-buffered PSUM tiles so exp(kc) overlaps QK(kc+1).
#   Normalization: reciprocal of denom row + gpsimd partition_broadcast +
#   DVE multiply. Output projection contracts ctxT over head dims in PSUM.
#
# kT/qT projections for groups 1-3 are emitted inside the attention loop so
# their PE work fills ScalarE-paced attention windows.
#
# Measured ~266-272us HW time per forward (all 8 cores, incl. DMA), vs
# ~384us cost-model estimate. Direct measurement shows ScalarE exp runs at
# ~189 Gelem/s from PSUM (1.8x faster than the cost-model formula), so the
# kernel is PE-streaming-bound: ~273us of pure column streaming (1280 MM
# slots x 512 cols @ 2.4GHz, QK row-pairs counted once) + LDWEIGHTS
# exposure + DMA ramp/tail. Out-proj loads each ctxT slice once and streams
# both output halves to halve its LDWEIGHTS on the serial tail.
#
# Alternatives measured and rejected (see kernel_v2.py + memory notes):
# column-tiled AV (M=64+64 via tile_position) IS concurrent on HW and cuts
# the core to 239us, but losing the ones-column forces an explicit softmax
# denominator whose cheapest correct form (bf16 DVE add-tree + ones-matmul
# reduce/replicate) re-exposes ~42us -> 281us total. fp8/DoubleRow fails
# the 2e-2 accuracy gate (e4m3 sim: 2.7e-2+). The M=65 ones-column design
# keeps every non-PE engine hidden under the PE roofline, which measures
# best end-to-end.

import contextlib

import numpy as np
import ml_dtypes

B, N, D, H = 4, 2048, 1024, 16
DH = D // H
OUT = 1024
NCORES = 8
KC = D // 128     # 8 contraction chunks for projections
RC = N // 128     # 16 row chunks
QC = N // 512     # 4 qrow chunks of 512
G = 4             # head-pair groups per core (8 heads / 2)
HPC = H // 2      # heads per core

_cache = {}


def _build_module(reps=1):
    import concourse.mybir as mybir
    import concourse.tile as tile
    from concourse import bacc

    bf16 = mybir.dt.bfloat16
    f32 = mybir.dt.float32
    Exp = mybir.ActivationFunctionType.Exp
    MULT = mybir.AluOpType.mult
    ADD = mybir.AluOpType.add

    nc = bacc.Bacc(trn_type="TRN2", target_bir_lowering=False)

    xt_q = nc.declare_dram_parameter("xt_q", [KC, 128, N], bf16, isOutput=False)
    xt_k = nc.declare_dram_parameter("xt_k", [KC, 128, N], bf16, isOutput=False)
    xt_v = nc.declare_dram_parameter("xt_v", [KC, 128, N], bf16, isOutput=False)
    wq_d = nc.declare_dram_parameter("wq", [KC, 128, 512], bf16, isOutput=False)
    wk_d = nc.declare_dram_parameter("wk", [KC, 128, 512], bf16, isOutput=False)
    wv_d = nc.declare_dram_parameter("wv", [KC, 128, 512], bf16, isOutput=False)
    wo_d = nc.declare_dram_parameter("wo", [G, 128, OUT], bf16, isOutput=False)
    bq_d = nc.declare_dram_parameter("bq2", [G, 128, 1], f32, isOutput=False)
    bk_d = nc.declare_dram_parameter("bk2", [G, 128, 1], f32, isOutput=False)
    bv_d = nc.declare_dram_parameter("bv_rep", [128, 512], f32, isOutput=False)
    bo_d = nc.declare_dram_parameter("bo_rep", [128, OUT], f32, isOutput=False)
    out_d = nc.declare_dram_parameter("out", [N, OUT], f32, isOutput=True)

    with tile.TileContext(nc) as tc, contextlib.ExitStack() as ctx:
        weights = ctx.enter_context(tc.tile_pool(name="weights", bufs=1))
        qkv = ctx.enter_context(tc.tile_pool(name="qkv", bufs=1))
        xt_pool = ctx.enter_context(tc.tile_pool(name="xt", bufs=16))
        expp = ctx.enter_context(tc.tile_pool(name="expp", bufs=5))
        ctxp = ctx.enter_context(tc.tile_pool(name="ctxp", bufs=1))
        small = ctx.enter_context(tc.tile_pool(name="small", bufs=4))
        outp = ctx.enter_context(tc.tile_pool(name="outp", bufs=3))
        ps_proj = ctx.enter_context(tc.tile_pool(name="ps_proj", bufs=2, space="PSUM"))
        ps_qk = ctx.enter_context(tc.tile_pool(name="ps_qk", bufs=2, space="PSUM"))
        ps_av = ctx.enter_context(tc.tile_pool(name="ps_av", bufs=2, space="PSUM"))

        for rep in range(reps):
            # ---- compute-critical DMAs first: v weights + xt_v, then xt_k
            # streams in during the v projection, then the remaining weights.
            # Weight blocks are DMA'd per-chunk so the first matmuls can start
            # as soon as chunk 0 lands.
            wv_sb = weights.tile([128, KC, 512], bf16, tag="wv")
            for kc in range(KC):
                nc.sync.dma_start(wv_sb[:, kc, :], wv_d[kc])
            bv_sb = weights.tile([128, 512], f32, tag="bv")
            nc.sync.dma_start(bv_sb[:], bv_d[:])
            xtv = []
            for kc in range(KC):
                t = xt_pool.tile([128, N], bf16, tag="xt")
                nc.sync.dma_start(t[:], xt_v[kc])
                xtv.append(t)
            wk_sb = weights.tile([128, KC, 512], bf16, tag="wk")
            for kc in range(KC):
                nc.sync.dma_start(wk_sb[:, kc, :], wk_d[kc])
            bk_sb = weights.tile([128, G, 1], f32, tag="bk")
            nc.sync.dma_start(bk_sb[:], bk_d.rearrange("g p o -> p g o"))

            # ---- V projection: v1[rc] = [v(64 cols per head) | 1] per head, bf16
            v1 = []
            for rc in range(RC):
                ps = ps_proj.tile([128, 512], f32, tag="pp")
                for kc in range(KC):
                    nc.tensor.matmul(
                        ps[:],
                        xtv[kc][:, rc * 128:(rc + 1) * 128],
                        wv_sb[:, kc, :],
                        start=(kc == 0), stop=(kc == KC - 1),
                    )
                t = qkv.tile([128, HPC, DH + 1], bf16, tag=f"v1_{rc}")
                nc.vector.memset(t[:], 1.0)
                nc.vector.tensor_tensor(
                    t[:, :, 0:DH],
                    ps.rearrange("p (h d) -> p h d", h=HPC),
                    bv_sb.rearrange("p (h d) -> p h d", h=HPC),
                    ADD,
                )
                v1.append(t)

            # ---- K^T projection for all 4 pair-groups: kT[g] [128(2 heads x 64), N]
            xtk = []
            for kc in range(KC):
                t = xt_pool.tile([128, N], bf16, tag="xt")
                nc.sync.dma_start(t[:], xt_k[kc])
                xtk.append(t)
            wq_sb = weights.tile([128, KC, 512], bf16, tag="wq")
            for kc in range(KC):
                nc.sync.dma_start(wq_sb[:, kc, :], wq_d[kc])
            bq_sb = weights.tile([128, G, 1], f32, tag="bq")
            nc.sync.dma_start(bq_sb[:], bq_d.rearrange("g p o -> p g o"))
            def emit_kT(g):
                t = qkv.tile([128, N], bf16, tag=f"kT_{g}", name=f"kT_{g}")
                for qn2 in range(0, QC, 2):
                    pss = [ps_proj.tile([128, 512], f32, tag="pp", name=f"pk{g}{qn2}{j}")
                           for j in range(2)]
                    for kc in range(KC):
                        for j in range(2):
                            nc.tensor.matmul(
                                pss[j][:],
                                wk_sb[:, kc, g * 128:(g + 1) * 128],
                                xtk[kc][:, (qn2 + j) * 512:(qn2 + j + 1) * 512],
                                start=(kc == 0), stop=(kc == KC - 1),
                            )
                    for j in range(2):
                        nc.vector.tensor_scalar_add(
                            t[:, (qn2 + j) * 512:(qn2 + j + 1) * 512],
                            pss[j][:], bk_sb[:, g, :]
                        )
                return t

            kT = [None] * G
            kT[0] = emit_kT(0)

            # ---- per pair-group: Q^T projection then attention
            xtq = []
            for kc in range(KC):
                t = xt_pool.tile([128, N], bf16, tag="xt")
                nc.sync.dma_start(t[:], xt_q[kc])
                xtq.append(t)

            wo_sb = weights.tile([128, G, OUT], bf16, tag="wo")
            for g in range(G):
                nc.sync.dma_start(wo_sb[:, g, :], wo_d[g])
            bo_sb = weights.tile([128, OUT], f32, tag="bo")
            nc.sync.dma_start(bo_sb[:], bo_d[:])
            ctxT = [
                ctxp.tile([128, N], bf16, tag=f"ctxT_{g}", name=f"ctxT_{g}")
                for g in range(G)
            ]
            for g in range(G):
                if g > 0:
                    kT[g] = emit_kT(g)
                qT = qkv.tile([128, N], bf16, tag=f"qT_{g}", name=f"qT_{g}")
                for qn2 in range(0, QC, 2):
                    pss = [ps_proj.tile([128, 512], f32, tag="pp", name=f"pq{g}{qn2}{j}")
                           for j in range(2)]
                    for kc in range(KC):
                        for j in range(2):
                            nc.tensor.matmul(
                                pss[j][:],
                                wq_sb[:, kc, g * 128:(g + 1) * 128],
                                xtq[kc][:, (qn2 + j) * 512:(qn2 + j + 1) * 512],
                                start=(kc == 0), stop=(kc == KC - 1),
                            )
                    for j in range(2):
                        nc.vector.tensor_scalar_add(
                            qT[:, (qn2 + j) * 512:(qn2 + j + 1) * 512],
                            pss[j][:], bq_sb[:, g, :]
                        )

                for qc in range(QC):
                    av_lo = ps_av.tile([DH + 1, 512], f32, tag="av")
                    av_hi = ps_av.tile([DH + 1, 512], f32, tag="av")
                    for kcc in range(RC):
                        # scoresT: lhsT = kT slice (K=64), row-tiled pair (heads 2g, 2g+1)
                        pp = ps_qk.tile([128, 1024], f32, tag="qk")
                        nc.tensor.matmul(
                            pp[:, 0:512],
                            kT[g][0:64, kcc * 128:(kcc + 1) * 128],
                            qT[0:64, qc * 512:(qc + 1) * 512],
                            start=True, stop=True,
                        )
                        nc.tensor.matmul(
                            pp[:, 512:1024],
                            kT[g][64:128, kcc * 128:(kcc + 1) * 128],
                            qT[64:128, qc * 512:(qc + 1) * 512],
                            start=True, stop=True,
                        )
                        eT = expp.tile([128, 1024], bf16, tag="exp")
                        nc.scalar.activation(eT[:], pp[:], Exp)
                        nc.tensor.matmul(
                            av_lo[:],
                            v1[kcc][:, 2 * g, :],
                            eT[:, 0:512],
                            start=(kcc == 0), stop=(kcc == RC - 1),
                        )
                        nc.tensor.matmul(
                            av_hi[:],
                            v1[kcc][:, 2 * g + 1, :],
                            eT[:, 512:1024],
                            start=(kcc == 0), stop=(kcc == RC - 1),
                        )
                    # copy raw ctxT+denom out of PSUM fast (releases the AV slot),
                    # then normalize off the critical path
                    raw_lo = small.tile([DH + 1, 512], f32, tag="raw")
                    nc.vector.tensor_copy(raw_lo[:], av_lo[:])
                    raw_hi = small.tile([DH + 1, 512], f32, tag="raw")
                    nc.vector.tensor_copy(raw_hi[:], av_hi[:])
                    # normalize head lo -> ctxT[g][0:64, qc block]
                    r1 = small.tile([1, 512], f32, tag="r1", bufs=2)
                    nc.vector.reciprocal(r1[:], raw_lo[DH:DH + 1, :])
                    rb = small.tile([64, 512], f32, tag="rb", bufs=2)
                    nc.gpsimd.partition_broadcast(rb[:], r1[:])
                    nc.vector.tensor_tensor(
                        ctxT[g][0:64, qc * 512:(qc + 1) * 512],
                        raw_lo[0:DH, :], rb[:], MULT,
                    )
                    # normalize head hi -> bounce tile, DMA into partitions 64:128
                    r1b = small.tile([1, 512], f32, tag="r1", bufs=2)
                    nc.vector.reciprocal(r1b[:], raw_hi[DH:DH + 1, :])
                    rbb = small.tile([64, 512], f32, tag="rb", bufs=2)
                    nc.gpsimd.partition_broadcast(rbb[:], r1b[:])
                    tmp = small.tile([64, 512], bf16, tag="tmp", bufs=2)
                    nc.vector.tensor_tensor(tmp[:], raw_hi[0:DH, :], rbb[:], MULT)
                    nc.sync.dma_start(ctxT[g][64:128, qc * 512:(qc + 1) * 512], tmp[:])


            # ---- output projection: out = ctx @ Wo_slice (+ bo on even cores)
            # g outer / ncol inner: each ctxT slice is loaded as PE weights
            # once and streams both 512-wide output halves.
            for rc in range(RC):
                pss = [ps_proj.tile([128, 512], f32, tag="pp", name=f"po{rc}{n}")
                       for n in range(2)]
                for g in range(G):
                    for ncol in range(2):
                        nc.tensor.matmul(
                            pss[ncol][:],
                            ctxT[g][:, rc * 128:(rc + 1) * 128],
                            wo_sb[:, g, ncol * 512:(ncol + 1) * 512],
                            start=(g == 0), stop=(g == G - 1),
                        )
                for ncol in range(2):
                    ob = outp.tile([128, 512], f32, tag="ob")
                    nc.vector.tensor_tensor(
                        ob[:], pss[ncol][:], bo_sb[:, ncol * 512:(ncol + 1) * 512], ADD
                    )
                    nc.sync.dma_start(
                        out_d[rc * 128:(rc + 1) * 128,
                              ncol * 512:(ncol + 1) * 512], ob[:]
                    )

    nc.compile()
    return nc


def _get_module(reps=1):
    key = ("nc", reps)
    if key not in _cache:
        _cache[key] = _build_module(reps)
    return _cache[key]


def _get_runner(reps=1, donate=True):
    """Build the PJRT executable once (mirrors bass2jax.run_bass_via_pjrt) and
    return a callable in_maps -> list of per-core output dicts."""
    rkey = ("runner", reps, donate)
    if rkey in _cache:
        return _cache[rkey]

    import jax
    import numpy as np
    import concourse.mybir as mybir
    from concourse import bass2jax
    from jax.sharding import Mesh, PartitionSpec
    from jax.experimental.shard_map import shard_map

    nc = _get_module(reps)
    bass2jax.install_neuronx_cc_hook()

    partition_name = nc.partition_id_tensor.name if nc.partition_id_tensor else None
    in_names, out_names, out_avals, zero_outs = [], [], [], []
    for alloc in nc.m.functions[0].allocations:
        if not isinstance(alloc, mybir.MemoryLocationSet):
            continue
        name = alloc.memorylocations[0].name
        if alloc.kind == "ExternalInput":
            if name != partition_name:
                in_names.append(name)
        elif alloc.kind == "ExternalOutput":
            shape = tuple(alloc.tensor_shape)
            dtype = mybir.dt.np(alloc.dtype)
            out_names.append(name)
            out_avals.append(jax.core.ShapedArray(shape, dtype))
            zero_outs.append(np.zeros(shape, dtype))
    n_params = len(in_names)
    n_outs = len(out_avals)
    all_in_names = list(in_names) + list(out_names)
    if partition_name is not None:
        all_in_names.append(partition_name)
    donate_idx = tuple(range(n_params, n_params + n_outs))

    def _body(*args):
        operands = list(args)
        if partition_name is not None:
            operands.append(bass2jax.partition_id_tensor())
        outs = bass2jax._bass_exec_p.bind(
            *operands,
            out_avals=tuple(out_avals),
            in_names=tuple(all_in_names),
            out_names=tuple(out_names),
            lowering_input_output_aliases=(),
            sim_require_finite=True,
            sim_require_nnan=True,
            nc=nc,
        )
        return tuple(outs)

    devices = jax.devices()[:NCORES]
    mesh = Mesh(np.asarray(devices), ("core",))
    in_specs = (PartitionSpec("core"),) * (n_params + n_outs)
    out_specs = (PartitionSpec("core"),) * n_outs
    sharded = jax.jit(
        shard_map(_body, mesh=mesh, in_specs=in_specs, out_specs=out_specs,
                  check_rep=False),
        donate_argnums=(donate_idx if donate else ()), keep_unused=True,
    )

    def run(in_maps):
        concat_in = [
            np.concatenate([np.asarray(in_maps[c][name]) for c in range(NCORES)], axis=0)
            for name in in_names
        ]
        concat_zeros = [
            np.zeros((NCORES * z.shape[0], *z.shape[1:]), z.dtype) for z in zero_outs
        ]
        out_arrs = sharded(*concat_in, *concat_zeros)
        return [
            {
                name: np.asarray(out_arrs[i]).reshape(NCORES, *out_avals[i].shape)[c]
                for i, name in enumerate(out_names)
            }
            for c in range(NCORES)
        ]

    run.in_names = in_names
    run.out_names = out_names
    run.out_avals = out_avals
    run.zero_outs = zero_outs
    run.sharded = sharded
    _cache[rkey] = run
    return run


def _shard_inputs(key, value, query, Wk, bk, Wv, bv, Wq, bq, Wo, bo):
    bf = ml_dtypes.bfloat16
    f32 = np.float32
    scale = 1.0 / np.sqrt(np.float32(DH))

    xt = {}  # per batch transposed inputs
    for b in range(B):
        xt[b] = {
            "q": np.ascontiguousarray(query[b].T).reshape(KC, 128, N).astype(bf),
            "k": np.ascontiguousarray(key[b].T).reshape(KC, 128, N).astype(bf),
            "v": np.ascontiguousarray(value[b].T).reshape(KC, 128, N).astype(bf),
        }

    in_maps = []
    for c in range(NCORES):
        b, half = divmod(c, 2)
        cols = slice(half * 512, (half + 1) * 512)
        in_maps.append({
            "xt_q": xt[b]["q"],
            "xt_k": xt[b]["k"],
            "xt_v": xt[b]["v"],
            "wq": np.ascontiguousarray(Wq[:, cols] * scale).reshape(KC, 128, 512).astype(bf),
            "wk": np.ascontiguousarray(Wk[:, cols]).reshape(KC, 128, 512).astype(bf),
            "wv": np.ascontiguousarray(Wv[:, cols]).reshape(KC, 128, 512).astype(bf),
            "wo": np.ascontiguousarray(Wo[cols, :]).reshape(G, 128, OUT).astype(bf),
            "bq2": (bq[cols] * scale).reshape(G, 128, 1).astype(f32),
            "bk2": bk[cols].reshape(G, 128, 1).astype(f32),
            "bv_rep": np.broadcast_to(bv[cols], (128, 512)).astype(f32),
            "bo_rep": (np.broadcast_to(bo, (128, OUT)).astype(f32)
                       if half == 0 else np.zeros((128, OUT), f32)),
        })
    return in_maps


def kernel(key, value, query, Wk, bk, Wv, bv, Wq, bq, Wo, bo):
    key, value, query = np.asarray(key), np.asarray(value), np.asarray(query)
    Wk, bk, Wv, bv = np.asarray(Wk), np.asarray(bk), np.asarray(Wv), np.asarray(bv)
    Wq, bq, Wo, bo = np.asarray(Wq), np.asarray(bq), np.asarray(Wo), np.asarray(bo)

    run = _get_runner()
    in_maps = _shard_inputs(key, value, query, Wk, bk, Wv, bv, Wq, bq, Wo, bo)
    results = run(in_maps)
    parts = [results[c]["out"] for c in range(NCORES)]
    out = np.empty((B, N, OUT), np.float32)
    for b in range(B):
        np.add(parts[2 * b], parts[2 * b + 1], out=out[b])
    return out



# revision 2
# speedup vs baseline: 1.2409x; 1.2409x over previous
# Multi-head attention (B=4, N=2048, D=1024, H=16, DH=64, OUT=1024) on 8 TRN2 NeuronCores.
#
# v2: paired AV matmuls + explicit softmax denominators off the PE.
#
# Sharding: 4 groups x 2 cores. Core c -> batch c//2, head-half c%2 (8 heads).
# Wq/Wk/Wv column-split per head group, Wo row-split; host sums the two
# partial outputs per batch (row-parallel unshard). bo folded in on even cores.
#
# Differences vs v1 (the M=65 ones-column design):
#   - AV matmuls are column-paired: head-lo writes PSUM partitions 0:64
#     (PE col group 0), head-hi partitions 64:128 (col group 64); both stream
#     their own 512-col eT half concurrently -> AV drops from 512 to 256
#     512-col PE slots. Total PE streaming 1024 slots (~218us) vs 1280 (~273us).
#   - Softmax denominators: DVE accumulates eT tiles (fp16, 2x/4x modes)
#     per kcc; gpsimd partition_all_reduce folds 128 partitions (and
#     broadcasts); reciprocal_approx_fast on DVE; normalization multiplies
#     straight out of the AV PSUM into ctxT.
#   - fp16 everywhere on the data path (better mantissa than bf16, same PE
#     rate); exp emitted with bias=-2.0 so e^(s-2) keeps fp16 range headroom
#     (measured scores in [-9.1, 8.2], denoms <= 14.7K).
#   - Output DMA'd as fp16 (host upcasts + sums halves).

import contextlib

import numpy as np

B, N, D, H = 4, 2048, 1024, 16
DH = D // H
OUT = 1024
NCORES = 8
KC = D // 128     # 8 contraction chunks for projections
RC = N // 128     # 16 row chunks
QC = N // 512     # 4 qrow chunks of 512
G = 4             # head-pair groups per core (8 heads / 2)
HPC = H // 2      # heads per core

EXP_BIAS = -2.0   # exp(s + EXP_BIAS): cancels in softmax, guards fp16 range

import os
PROBE = ""  # probe hooks disabled in the final kernel
# of the 16 kcc tiles per attention block, how many get their exp computed
# on DVE via the Schraudolph int16/bf16-bits trick instead of ScalarE
N_SCHRAUD = 0  # DVE fast-exp offload: no speedup at equilibrium, keep exact
N_GPADDS = 0  # gpsimd dn adds measured slower (serial chain)

_cache = {}


def _build_module(reps=1):
    import concourse.mybir as mybir
    import concourse.tile as tile
    from concourse import bacc, bass_isa

    f16 = mybir.dt.float16
    f32 = mybir.dt.float32
    bf16 = mybir.dt.bfloat16
    i16 = mybir.dt.int16
    Exp = mybir.ActivationFunctionType.Exp
    MULT = mybir.AluOpType.mult
    ADD = mybir.AluOpType.add

    import math
    # Schraudolph on bf16 bit patterns: int16 y = A*(s + EXP_BIAS) + 16256 + C
    # viewed as bf16 approximates exp(s + EXP_BIAS) with ~+-3% PWL error
    SCH_A = 128.0 / math.log(2.0)
    SCH_B = 127 * 128 + SCH_A * EXP_BIAS - 6.5
    # which kcc tiles go to DVE (avoid 0: prev_eT bookkeeping)
    SCHRAUD_KCC = {5, 11, 8, 3, 13, 7, 2, 10}
    sch_set = set()
    if N_SCHRAUD:
        sch_set = set(list({5, 11, 8, 3, 13, 7, 2, 10})[:0])  # placeholder
        sch_set = {5, 11, 8, 3, 13, 7, 2, 10}
        sch_set = set(sorted(sch_set)[:N_SCHRAUD])
    gp_set = set()
    if N_GPADDS:
        gp_set = set(x for x in (4, 9, 14, 6, 12) if x not in sch_set)
        gp_set = set(sorted(gp_set)[:N_GPADDS])

    nc = bacc.Bacc(trn_type="TRN2", target_bir_lowering=False)

    xt_q = nc.declare_dram_parameter("xt_q", [KC, 128, N], f16, isOutput=False)
    xt_k = nc.declare_dram_parameter("xt_k", [KC, 128, N], f16, isOutput=False)
    xt_v = nc.declare_dram_parameter("xt_v", [KC, 128, N], f16, isOutput=False)
    wq_d = nc.declare_dram_parameter("wq", [KC, 128, 512], f16, isOutput=False)
    wk_d = nc.declare_dram_parameter("wk", [KC, 128, 512], f16, isOutput=False)
    wv_d = nc.declare_dram_parameter("wv", [KC, 128, 512], f16, isOutput=False)
    wo_d = nc.declare_dram_parameter("wo", [G, 128, OUT], f16, isOutput=False)
    bq_d = nc.declare_dram_parameter("bq2", [G, 128, 1], f32, isOutput=False)
    bk_d = nc.declare_dram_parameter("bk2", [G, 128, 1], f32, isOutput=False)
    bv_d = nc.declare_dram_parameter("bv_rep", [128, 512], f32, isOutput=False)
    bo_d = nc.declare_dram_parameter("bo_rep", [128, OUT], f32, isOutput=False)
    out_d = nc.declare_dram_parameter("out", [N, OUT], f16, isOutput=True)

    with tile.TileContext(nc) as tc, contextlib.ExitStack() as ctx:
        weights = ctx.enter_context(tc.tile_pool(name="weights", bufs=1))
        qkv = ctx.enter_context(tc.tile_pool(name="qkv", bufs=1))
        xt_pool = ctx.enter_context(tc.tile_pool(name="xt", bufs=16))
        expp = ctx.enter_context(tc.tile_pool(name="expp", bufs=5))
        ctxp = ctx.enter_context(tc.tile_pool(name="ctxp", bufs=1))
        small = ctx.enter_context(tc.tile_pool(name="small", bufs=4))
        outp = ctx.enter_context(tc.tile_pool(name="outp", bufs=3))
        ps_proj = ctx.enter_context(tc.tile_pool(name="ps_proj", bufs=2, space="PSUM"))
        ps_qk = ctx.enter_context(tc.tile_pool(name="ps_qk", bufs=2, space="PSUM"))
        ps_av = ctx.enter_context(tc.tile_pool(name="ps_av", bufs=2, space="PSUM"))

        for rep in range(reps):
            # ---- compute-critical DMAs first: v weights + xt_v, then xt_k
            # streams in during the v projection, then the remaining weights.
            wv_sb = weights.tile([128, KC, 512], f16, tag="wv")
            for kc in range(KC):
                nc.sync.dma_start(wv_sb[:, kc, :], wv_d[kc])
            bv_sb = weights.tile([128, 512], f32, tag="bv")
            nc.sync.dma_start(bv_sb[:], bv_d[:])
            xtv = []
            for kc in range(KC):
                t = xt_pool.tile([128, N], f16, tag="xt")
                nc.sync.dma_start(t[:], xt_v[kc])
                xtv.append(t)
            wk_sb = weights.tile([128, KC, 512], f16, tag="wk")
            for kc in range(KC):
                nc.sync.dma_start(wk_sb[:, kc, :], wk_d[kc])
            bk_sb = weights.tile([128, G, 1], f32, tag="bk")
            nc.sync.dma_start(bk_sb[:], bk_d.rearrange("g p o -> p g o"))
            ebias = weights.tile([128, 1], f32, tag="ebias")
            nc.vector.memset(ebias[:], EXP_BIAS)
            ones_sb = weights.tile([128, 64], f16, tag="ones")
            nc.vector.memset(ones_sb[:], 1.0)

            # ---- V projection: v1[rc] [128, 8 heads, 64] fp16
            v1 = []
            for rc in range(RC):
                ps = ps_proj.tile([128, 512], f32, tag="pp")
                for kc in range(KC):
                    nc.tensor.matmul(
                        ps[:],
                        xtv[kc][:, rc * 128:(rc + 1) * 128],
                        wv_sb[:, kc, :],
                        start=(kc == 0), stop=(kc == KC - 1),
                    )
                t = qkv.tile([128, HPC, DH], f16, tag=f"v1_{rc}")
                nc.vector.tensor_tensor(
                    t[:],
                    ps.rearrange("p (h d) -> p h d", h=HPC),
                    bv_sb.rearrange("p (h d) -> p h d", h=HPC),
                    ADD,
                )
                v1.append(t)

            # ---- K^T projection per pair-group: kT[g] [128(2 heads x 64), N]
            xtk = []
            for kc in range(KC):
                t = xt_pool.tile([128, N], f16, tag="xt")
                nc.sync.dma_start(t[:], xt_k[kc])
                xtk.append(t)
            wq_sb = weights.tile([128, KC, 512], f16, tag="wq")
            for kc in range(KC):
                nc.sync.dma_start(wq_sb[:, kc, :], wq_d[kc])
            bq_sb = weights.tile([128, G, 1], f32, tag="bq")
            nc.sync.dma_start(bq_sb[:], bq_d.rearrange("g p o -> p g o"))

            def emit_kT(g):
                t = qkv.tile([128, N], f16, tag="kT", bufs=2, name=f"kT_{g}")
                for qn2 in range(0, QC, 2):
                    pss = [ps_proj.tile([128, 512], f32, tag="pp", name=f"pk{g}{qn2}{j}")
                           for j in range(2)]
                    for kc in range(KC):
                        for j in range(2):
                            nc.tensor.matmul(
                                pss[j][:],
                                wk_sb[:, kc, g * 128:(g + 1) * 128],
                                xtk[kc][:, (qn2 + j) * 512:(qn2 + j + 1) * 512],
                                start=(kc == 0), stop=(kc == KC - 1),
                            )
                    for j in range(2):
                        nc.vector.tensor_scalar_add(
                            t[:, (qn2 + j) * 512:(qn2 + j + 1) * 512],
                            pss[j][:], bk_sb[:, g, :]
                        )
                return t

            kT = [None] * G
            kT[0] = emit_kT(0)

            # ---- per pair-group: Q^T projection then attention
            xtq = []
            for kc in range(KC):
                t = xt_pool.tile([128, N], f16, tag="xt")
                nc.sync.dma_start(t[:], xt_q[kc])
                xtq.append(t)

            wo_sb = weights.tile([128, G, OUT], f16, tag="wo")
            for g in range(G):
                nc.sync.dma_start(wo_sb[:, g, :], wo_d[g])
            bo_sb = weights.tile([128, OUT], f32, tag="bo")
            nc.sync.dma_start(bo_sb[:], bo_d[:])
            ctxT = [
                ctxp.tile([128, N], f16, tag=f"ctxT_{g}", name=f"ctxT_{g}")
                for g in range(G)
            ]
            for g in range(G):
                if g > 0:
                    kT[g] = emit_kT(g)
                qT = qkv.tile([128, N], f16, tag="qT", bufs=2, name=f"qT_{g}")
                for qn2 in range(0, QC, 2):
                    pss = [ps_proj.tile([128, 512], f32, tag="pp", name=f"pq{g}{qn2}{j}")
                           for j in range(2)]
                    for kc in range(KC):
                        for j in range(2):
                            nc.tensor.matmul(
                                pss[j][:],
                                wq_sb[:, kc, g * 128:(g + 1) * 128],
                                xtq[kc][:, (qn2 + j) * 512:(qn2 + j + 1) * 512],
                                start=(kc == 0), stop=(kc == KC - 1),
                            )
                    for j in range(2):
                        nc.vector.tensor_scalar_add(
                            qT[:, (qn2 + j) * 512:(qn2 + j + 1) * 512],
                            pss[j][:], bq_sb[:, g, :]
                        )

                if PROBE == "noexp":
                    ejunk = qkv.tile([128, 1024], f16, tag="ejunk")
                    nc.vector.memset(ejunk[:], 0.001)
                def emit_outproj_chunk(qcd):
                    # out-proj for token rows 4*qcd..4*qcd+3 (needs all
                    # groups' ctxT for those rows to be final)
                    for rc in range(4 * qcd, 4 * qcd + 4):
                        pss = [ps_proj.tile([128, 512], f32, tag="pp",
                                            name=f"po{rc}{n}")
                               for n in range(2)]
                        for gg in range(G):
                            for ncol in range(2):
                                nc.tensor.matmul(
                                    pss[ncol][:],
                                    ctxT[gg][:, rc * 128:(rc + 1) * 128],
                                    wo_sb[:, gg, ncol * 512:(ncol + 1) * 512],
                                    start=(gg == 0), stop=(gg == G - 1),
                                )
                        for ncol in range(2):
                            ob = outp.tile([128, 512], f16, tag="ob")
                            nc.vector.tensor_tensor(
                                ob[:], pss[ncol][:],
                                bo_sb[:, ncol * 512:(ncol + 1) * 512], ADD
                            )
                            nc.sync.dma_start(
                                out_d[rc * 128:(rc + 1) * 128,
                                      ncol * 512:(ncol + 1) * 512], ob[:]
                            )

                for qc in range(QC):
                    av = ps_av.tile([128, 512], f32, tag="av")
                    dn = small.tile([128, 1024], f16, tag="dn", bufs=2)
                    prev_eT = None
                    pp = None
                    for kcc in range(RC):
                        # scoresT: lhsT = kT slice (K=64), row-group pair
                        pp = ps_qk.tile([128, 1024], f32, tag="qk")
                        nc.tensor.matmul(
                            pp[:, 0:512],
                            kT[g][0:64, kcc * 128:(kcc + 1) * 128],
                            qT[0:64, qc * 512:(qc + 1) * 512],
                            start=True, stop=True,
                        )
                        nc.tensor.matmul(
                            pp[:, 512:1024],
                            kT[g][64:128, kcc * 128:(kcc + 1) * 128],
                            qT[64:128, qc * 512:(qc + 1) * 512],
                            start=True, stop=True,
                        )
                        if PROBE == "noexp":
                            eT = ejunk
                        elif kcc in sch_set:
                            # DVE fast-exp: int16(A*s + B) bits viewed as bf16
                            eTi = expp.tile([128, 1024], i16, tag="exps")
                            nc.vector.tensor_scalar(
                                eTi[:], pp[:], SCH_A, SCH_B, MULT, ADD
                            )
                            eT = None
                            eview = lambda sl: eTi[:, sl].bitcast(bf16)
                        else:
                            eT = expp.tile([128, 1024], f16, tag="exp")
                            nc.scalar.activation(eT[:], pp[:], Exp, bias=ebias[:])
                            eview = lambda sl, _t=eT: _t[:, sl]
                        # paired AV: head-lo -> PE col group 0 (PSUM parts
                        # 0:64), head-hi -> col group 64 (parts 64:128)
                        nc.tensor.matmul(
                            av[0:64, :],
                            v1[kcc][:, 2 * g, :],
                            eview(slice(0, 512)),
                            start=(kcc == 0), stop=(kcc == RC - 1),
                        )
                        nc.tensor.matmul(
                            av[64:128, :],
                            v1[kcc][:, 2 * g + 1, :],
                            eview(slice(512, 1024)),
                            start=(kcc == 0), stop=(kcc == RC - 1),
                        )
                        # denominator partial accumulate on DVE (fp16)
                        if PROBE in ("nodenom", "noexp"):
                            pass
                        elif kcc == 0:
                            prev_eT = eT
                        elif kcc == 1:
                            nc.vector.tensor_tensor(
                                dn[:], prev_eT[:], eview(slice(0, 1024)), ADD
                            )
                        else:
                            eng = nc.gpsimd if kcc in gp_set else nc.vector
                            eng.tensor_tensor(
                                dn[:], dn[:], eview(slice(0, 1024)), ADD
                            )
                    if PROBE in ("nodenom", "noexp"):
                        r = bv_sb  # junk multiplier, timing only
                        nc.vector.tensor_tensor(
                            ctxT[g][0:64, qc * 512:(qc + 1) * 512],
                            av[0:64, :], r[0:64, 0:512], MULT,
                        )
                        nc.vector.tensor_tensor(
                            ctxT[g][64:128, qc * 512:(qc + 1) * 512],
                            av[64:128, :], r[64:128, 0:512], MULT,
                        )
                    else:
                        # cross-partition denominator reduce ON THE PE: one
                        # paired ones-matmul (213ns) into the just-freed QK
                        # PSUM tile; every output row = column sum, so the
                        # broadcast for normalization is free.
                        # parts 0:64 <- sums of head-lo cols, 64:128 head-hi
                        nc.tensor.matmul(
                            pp[0:64, 0:512], ones_sb[:, 0:64], dn[:, 0:512],
                            start=True, stop=True,
                        )
                        nc.tensor.matmul(
                            pp[64:128, 0:512], ones_sb[:, 0:64], dn[:, 512:1024],
                            start=True, stop=True,
                        )
                        r = small.tile([128, 512], f32, tag="recip", bufs=2)
                        nc.vector.reciprocal_approx_fast(r[:], pp[:, 0:512])
                        # normalize straight out of PSUM into ctxT
                        nc.vector.tensor_tensor(
                            ctxT[g][0:64, qc * 512:(qc + 1) * 512],
                            av[0:64, :], r[0:64, :], MULT,
                        )
                        nc.vector.tensor_tensor(
                            ctxT[g][64:128, qc * 512:(qc + 1) * 512],
                            av[64:128, :], r[64:128, :], MULT,
                        )
                    if g == G - 1 and qc > 0:
                        # lag by one block so the PE never waits on the
                        # denominator chain of the block just finished
                        emit_outproj_chunk(qc - 1)
            emit_outproj_chunk(QC - 1)

            # (output projection is interleaved into the g == G-1 blocks)

    nc.compile()
    return nc


def _get_module(reps=1):
    key = ("nc", reps)
    if key not in _cache:
        _cache[key] = _build_module(reps)
    return _cache[key]


def _get_runner(reps=1, donate=True):
    """Build the PJRT executable once (mirrors bass2jax.run_bass_via_pjrt) and
    return a callable in_maps -> list of per-core output dicts."""
    rkey = ("runner", reps, donate)
    if rkey in _cache:
        return _cache[rkey]

    import jax
    import numpy as np
    import concourse.mybir as mybir
    from concourse import bass2jax
    from jax.sharding import Mesh, PartitionSpec
    from jax.experimental.shard_map import shard_map

    nc = _get_module(reps)
    bass2jax.install_neuronx_cc_hook()

    partition_name = nc.partition_id_tensor.name if nc.partition_id_tensor else None
    in_names, out_names, out_avals, zero_outs = [], [], [], []
    for alloc in nc.m.functions[0].allocations:
        if not isinstance(alloc, mybir.MemoryLocationSet):
            continue
        name = alloc.memorylocations[0].name
        if alloc.kind == "ExternalInput":
            if name != partition_name:
                in_names.append(name)
        elif alloc.kind == "ExternalOutput":
            shape = tuple(alloc.tensor_shape)
            dtype = mybir.dt.np(alloc.dtype)
            out_names.append(name)
            out_avals.append(jax.core.ShapedArray(shape, dtype))
            zero_outs.append(np.zeros(shape, dtype))
    n_params = len(in_names)
    n_outs = len(out_avals)
    all_in_names = list(in_names) + list(out_names)
    if partition_name is not None:
        all_in_names.append(partition_name)
    donate_idx = tuple(range(n_params, n_params + n_outs))

    def _body(*args):
        operands = list(args)
        if partition_name is not None:
            operands.append(bass2jax.partition_id_tensor())
        outs = bass2jax._bass_exec_p.bind(
            *operands,
            out_avals=tuple(out_avals),
            in_names=tuple(all_in_names),
            out_names=tuple(out_names),
            lowering_input_output_aliases=(),
            sim_require_finite=True,
            sim_require_nnan=True,
            nc=nc,
        )
        return tuple(outs)

    devices = jax.devices()[:NCORES]
    mesh = Mesh(np.asarray(devices), ("core",))
    in_specs = (PartitionSpec("core"),) * (n_params + n_outs)
    out_specs = (PartitionSpec("core"),) * n_outs
    sharded = jax.jit(
        shard_map(_body, mesh=mesh, in_specs=in_specs, out_specs=out_specs,
                  check_rep=False),
        donate_argnums=(donate_idx if donate else ()), keep_unused=True,
    )

    def run(in_maps):
        concat_in = [
            np.concatenate([np.asarray(in_maps[c][name]) for c in range(NCORES)], axis=0)
            for name in in_names
        ]
        concat_zeros = [
            np.zeros((NCORES * z.shape[0], *z.shape[1:]), z.dtype) for z in zero_outs
        ]
        out_arrs = sharded(*concat_in, *concat_zeros)
        return [
            {
                name: np.asarray(out_arrs[i]).reshape(NCORES, *out_avals[i].shape)[c]
                for i, name in enumerate(out_names)
            }
            for c in range(NCORES)
        ]

    run.in_names = in_names
    run.out_names = out_names
    run.out_avals = out_avals
    run.zero_outs = zero_outs
    run.sharded = sharded
    _cache[rkey] = run
    return run


def _shard_inputs(key, value, query, Wk, bk, Wv, bv, Wq, bq, Wo, bo):
    f16 = np.float16
    f32 = np.float32
    scale = 1.0 / np.sqrt(np.float32(DH))

    xt = {}  # per batch transposed inputs
    for b in range(B):
        xt[b] = {
            "q": np.ascontiguousarray(query[b].T).reshape(KC, 128, N).astype(f16),
            "k": np.ascontiguousarray(key[b].T).reshape(KC, 128, N).astype(f16),
            "v": np.ascontiguousarray(value[b].T).reshape(KC, 128, N).astype(f16),
        }

    in_maps = []
    for c in range(NCORES):
        b, half = divmod(c, 2)
        cols = slice(half * 512, (half + 1) * 512)
        in_maps.append({
            "xt_q": xt[b]["q"],
            "xt_k": xt[b]["k"],
            "xt_v": xt[b]["v"],
            "wq": np.ascontiguousarray(Wq[:, cols] * scale).reshape(KC, 128, 512).astype(f16),
            "wk": np.ascontiguousarray(Wk[:, cols]).reshape(KC, 128, 512).astype(f16),
            "wv": np.ascontiguousarray(Wv[:, cols]).reshape(KC, 128, 512).astype(f16),
            "wo": np.ascontiguousarray(Wo[cols, :]).reshape(G, 128, OUT).astype(f16),
            "bq2": (bq[cols] * scale).reshape(G, 128, 1).astype(f32),
            "bk2": bk[cols].reshape(G, 128, 1).astype(f32),
            "bv_rep": np.broadcast_to(bv[cols], (128, 512)).astype(f32),
            "bo_rep": (np.broadcast_to(bo, (128, OUT)).astype(f32)
                       if half == 0 else np.zeros((128, OUT), f32)),
        })
    return in_maps


def kernel(key, value, query, Wk, bk, Wv, bv, Wq, bq, Wo, bo):
    key, value, query = np.asarray(key), np.asarray(value), np.asarray(query)
    Wk, bk, Wv, bv = np.asarray(Wk), np.asarray(bk), np.asarray(Wv), np.asarray(bv)
    Wq, bq, Wo, bo = np.asarray(Wq), np.asarray(bq), np.asarray(Wo), np.asarray(bo)

    run = _get_runner()
    in_maps = _shard_inputs(key, value, query, Wk, bk, Wv, bv, Wq, bq, Wo, bo)
    results = run(in_maps)
    parts = [results[c]["out"].astype(np.float32) for c in range(NCORES)]
    out = np.empty((B, N, OUT), np.float32)
    for b in range(B):
        np.add(parts[2 * b], parts[2 * b + 1], out=out[b])
    return out
